# revision 4
# baseline (speedup 1.0000x reference)
"""Causal multi-head attention (B=4, S=2048, D=1024, H=16) on 8 TRN2 NeuronCores.

Sharding: zero-collective. Core c handles batch b=c//2 and a causally-balanced
half of the queries: half 0 = sequence quarters {0,3}, half 1 = quarters {1,2}
(both halves have equal causal-attention work). Each core computes Q for its
1024 tokens, K/V for the full sequence of its batch (duplicated across the two
cores of a batch -- cheaper than any collective), all 16 heads of causal
attention, and the output projection for its tokens. Host reassembles.

All 8 cores run one SPMD graph; per-core differences live only in the DMA'd
data. The per-core key axis is permuted to [own tokens | other tokens] so the
causal diagonal lands at identical graph positions on every core; padding and
block-level causality enter through a per-core additive bias (per-partition,
folded into the ScalarE exp), and the within-diagonal triangle through a
constant 0/1 multiplicative mask.
"""

import os
import sys

sys.path.insert(0, "/opt/trn_rl_repo")

import numpy as np
import ml_dtypes

import concourse.bass as bass
import concourse.bacc as bacc
import concourse.tile as tile
from concourse import mybir
from concourse.bass_utils import run_bass_kernel_spmd

B, S, D, H = 4, 2048, 1024, 16
HD = D // H  # 64
P = 128
NQ = S // 2  # queries per core (1024)
KC = D // P  # 8 contraction chunks
NEG = -1e30
BF16 = mybir.dt.bfloat16
F32 = mybir.dt.float32
NPBF16 = ml_dtypes.bfloat16

# k-tile positions (in permuted token space) each slot's k-loop visits.
# slot 0 (lower own quarter, 512 q): own tiles 0-3 + first 4 rest tiles (8-11)
# slot 1 (upper own quarter): everything.
SLOT_KTILES = [[0, 1, 2, 3, 8, 9, 10, 11], list(range(16))]
# (slot, j) -> triangle shift for the diagonal band (j = loop index)
DIAG = {(0, 0): 0, (0, 1): 1, (0, 2): 2, (0, 3): 3,
        (1, 4): 0, (1, 5): 1, (1, 6): 2, (1, 7): 3}


def _build():
    nc = bacc.Bacc()

    xt = nc.declare_dram_parameter("xt", [P, KC, S], BF16, isOutput=False)
    wq = nc.declare_dram_parameter("wq", [P, KC, D], BF16, isOutput=False)
    wk = nc.declare_dram_parameter("wk", [P, KC, D], BF16, isOutput=False)
    wv = nc.declare_dram_parameter("wv", [P, KC, D], BF16, isOutput=False)
    wo = nc.declare_dram_parameter("wo", [P, KC, D], BF16, isOutput=False)
    bqp = nc.declare_dram_parameter("bqp", [P, KC], F32, isOutput=False)
    bkp = nc.declare_dram_parameter("bkp", [P, KC], F32, isOutput=False)
    bvr = nc.declare_dram_parameter("bvr", [P, D], F32, isOutput=False)
    bor = nc.declare_dram_parameter("bor", [P, D], F32, isOutput=False)
    pad = nc.declare_dram_parameter("pad", [P, 24], F32, isOutput=False)
    tri = nc.declare_dram_parameter("tri", [P, 4, 512], BF16, isOutput=False)
    out = nc.declare_dram_parameter("out", [NQ, D], F32, isOutput=True)

    from contextlib import ExitStack

    with tile.TileContext(nc) as tc, ExitStack() as ctx:
        wpool = ctx.enter_context(tc.tile_pool(name="wpool", bufs=1))
        xpool = ctx.enter_context(tc.tile_pool(name="xpool", bufs=2))
        bigpool = ctx.enter_context(tc.tile_pool(name="bigpool", bufs=1))
        epool = ctx.enter_context(tc.tile_pool(name="epool", bufs=6))
        spool = ctx.enter_context(tc.tile_pool(name="spool", bufs=3))
        pp_acc = ctx.enter_context(tc.tile_pool(name="pp_acc", bufs=2, space="PSUM"))
        pp_sc = ctx.enter_context(tc.tile_pool(name="pp_sc", bufs=2, space="PSUM"))
        pp_ctx = ctx.enter_context(tc.tile_pool(name="pp_ctx", bufs=2, space="PSUM"))
        pp_den = ctx.enter_context(tc.tile_pool(name="pp_den", bufs=2, space="PSUM"))

        # ---- constants into SBUF ----
        wq_s = wpool.tile([P, KC, D], BF16, tag="wq")
        wk_s = wpool.tile([P, KC, D], BF16, tag="wk")
        wv_s = wpool.tile([P, KC, D], BF16, tag="wv")
        wo_s = wpool.tile([P, KC, D], BF16, tag="wo")
        bq_s = wpool.tile([P, KC], F32, tag="bq")
        bk_s = wpool.tile([P, KC], F32, tag="bk")
        bv_s = wpool.tile([P, D], F32, tag="bv")
        bo_s = wpool.tile([P, D], F32, tag="bo")
        pad_s = wpool.tile([P, 24], F32, tag="pad")
        tri_s = wpool.tile([P, 4, 512], BF16, tag="tri")
        ones_s = wpool.tile([P, HD], BF16, tag="ones")
        for dst, src in ((wq_s, wq), (wk_s, wk), (wv_s, wv), (wo_s, wo),
                         (bq_s, bqp), (bk_s, bkp), (bv_s, bvr), (bo_s, bor),
                         (pad_s, pad), (tri_s, tri)):
            nc.sync.dma_start(dst[:], src[:])
        nc.vector.memset(ones_s[:], 1.0)

        # ---- big persistent activations ----
        qT_s = bigpool.tile([P, KC, NQ], BF16, tag="qT")     # [pairdims, pair, q]
        kT_s = bigpool.tile([P, KC, S], BF16, tag="kT")      # [pairdims, pair, k]
        v_s = bigpool.tile([P, S // P, D], BF16, tag="v")    # [k in tile, ktile, do]
        cT_s = bigpool.tile([P, KC, NQ], BF16, tag="cT")     # [pairdims, pair, q]

        def qkv_stage(st):
            """Project tokens [st*512, (st+1)*512) of the permuted axis."""
            ssl = slice(st * 512, (st + 1) * 512)
            xt_t = xpool.tile([P, KC, 512], BF16, tag="xt")
            nc.sync.dma_start(xt_t[:], xt[:, :, ssl])
            plan = [(wk_s, bk_s, kT_s)]
            if st < 2:  # own tokens -> also Q
                plan.append((wq_s, bq_s, qT_s))
            for w_s, b_s, dst in plan:
                for m in range(KC):
                    ps = pp_acc.tile([P, 512], F32, tag="acc")
                    for kc in range(KC):
                        nc.tensor.matmul(
                            ps[:], lhsT=w_s[:, kc, m * P:(m + 1) * P],
                            rhs=xt_t[:, kc, :],
                            start=(kc == 0), stop=(kc == KC - 1))
                    nc.vector.tensor_scalar_add(dst[:, m, ssl], ps[:], b_s[:, m:m + 1])
            for sub in range(4):
                for dt in range(2):
                    dsl = slice(dt * 512, (dt + 1) * 512)
                    ps = pp_acc.tile([P, 512], F32, tag="acc")
                    for kc in range(KC):
                        nc.tensor.matmul(
                            ps[:], lhsT=xt_t[:, kc, sub * P:(sub + 1) * P],
                            rhs=wv_s[:, kc, dsl],
                            start=(kc == 0), stop=(kc == KC - 1))
                    nc.vector.tensor_tensor(
                        v_s[:, st * 4 + sub, dsl], ps[:], bv_s[:, dsl],
                        mybir.AluOpType.add)

        def attention_slot(slot):
            qsl = slice(slot * 512, (slot + 1) * 512)
            ktiles = SLOT_KTILES[slot]
            nkt = len(ktiles)
            for pr in range(KC):  # head pair
                hA, hB = 2 * pr, 2 * pr + 1
                ctx_ps = pp_ctx.tile([P, 512], F32, tag="ctx")
                den_ps = pp_den.tile([P, 512], F32, tag="den")
                for j, m in enumerate(ktiles):
                    ksl = slice(m * P, (m + 1) * P)
                    scA = pp_sc.tile([P, 512], F32, tag="sc")
                    scB = pp_sc.tile([P, 512], F32, tag="sc")
                    nc.tensor.matmul(scA[:], lhsT=kT_s[0:HD, pr, ksl],
                                     rhs=qT_s[0:HD, pr, qsl],
                                     start=True, stop=True, tile_position=(0, 0))
                    nc.tensor.matmul(scB[:], lhsT=kT_s[HD:P, pr, ksl],
                                     rhs=qT_s[HD:P, pr, qsl],
                                     start=True, stop=True, tile_position=(HD, 0))
                    pcol = j if slot == 0 else 8 + j
                    eA = epool.tile([P, 512], BF16, tag="e")
                    eB = epool.tile([P, 512], BF16, tag="e")
                    nc.scalar.activation(eA[:], scA[:],
                                         mybir.ActivationFunctionType.Exp,
                                         bias=pad_s[:, pcol:pcol + 1], scale=0.125)
                    nc.scalar.activation(eB[:], scB[:],
                                         mybir.ActivationFunctionType.Exp,
                                         bias=pad_s[:, pcol:pcol + 1], scale=0.125)
                    sh = DIAG.get((slot, j))
                    if sh is not None:
                        nc.vector.tensor_tensor(eA[:], eA[:], tri_s[:, sh, :],
                                                mybir.AluOpType.mult)
                        nc.vector.tensor_tensor(eB[:], eB[:], tri_s[:, sh, :],
                                                mybir.AluOpType.mult)
                    st_, sp_ = (j == 0), (j == nkt - 1)
                    nc.tensor.matmul(ctx_ps[0:HD, :], lhsT=v_s[:, m, hA * HD:(hA + 1) * HD],
                                     rhs=eA[:], start=st_, stop=sp_, tile_position=(0, 0))
                    nc.tensor.matmul(ctx_ps[HD:P, :], lhsT=v_s[:, m, hB * HD:(hB + 1) * HD],
                                     rhs=eB[:], start=st_, stop=sp_, tile_position=(0, HD))
                    nc.tensor.matmul(den_ps[0:HD, :], lhsT=ones_s[:],
                                     rhs=eA[:], start=st_, stop=sp_, tile_position=(0, 0))
                    nc.tensor.matmul(den_ps[HD:P, :], lhsT=ones_s[:],
                                     rhs=eB[:], start=st_, stop=sp_, tile_position=(0, HD))
                rden = spool.tile([P, 512], F32, tag="rden")
                nc.vector.reciprocal(rden[:], den_ps[:])
                nc.vector.tensor_tensor(cT_s[:, pr, qsl], ctx_ps[:], rden[:],
                                        mybir.AluOpType.mult)

        def oproj(st8):
            osl = slice(st8 * P, (st8 + 1) * P)
            for dt in range(2):
                dsl = slice(dt * 512, (dt + 1) * 512)
                ps = pp_acc.tile([P, 512], F32, tag="acc")
                for kc in range(KC):
                    nc.tensor.matmul(ps[:], lhsT=cT_s[:, kc, osl],
                                     rhs=wo_s[:, kc, dsl],
                                     start=(kc == 0), stop=(kc == KC - 1))
                ob = spool.tile([P, 512], F32, tag="outsb")
                nc.vector.tensor_tensor(ob[:], ps[:], bo_s[:, dsl],
                                        mybir.AluOpType.add)
                nc.sync.dma_start(out[osl, dsl], ob[:])

        qkv_stage(0)
        qkv_stage(2)
        attention_slot(0)
        qkv_stage(1)
        qkv_stage(3)
        for st8 in range(4):
            oproj(st8)
        attention_slot(1)
        for st8 in range(4, 8):
            oproj(st8)

    nc.compile()
    return nc


def _core_inputs(c, x, padding_mask, Wq, bq, Wk, bk, Wv, bv, Wo, bo):
    b, h = c // 2, c % 2
    if h == 0:
        own = np.r_[0:512, 1536:2048]
        rest = np.r_[512:1536]
        qlo = (0, 1536)
    else:
        own = np.r_[512:1536]
        rest = np.r_[0:512, 1536:2048]
        qlo = (512, 1024)
    perm = np.concatenate([own, rest])

    xt = np.ascontiguousarray(
        x[b][perm].T.reshape(KC, P, S).transpose(1, 0, 2)).astype(NPBF16)

    def wl(W):
        return np.ascontiguousarray(
            W.T.reshape(KC, P, D).transpose(1, 0, 2)).astype(NPBF16)

    bqp = np.ascontiguousarray(bq.reshape(KC, P).T).astype(np.float32)
    bkp = np.ascontiguousarray(bk.reshape(KC, P).T).astype(np.float32)
    bvr = np.ascontiguousarray(np.tile(bv[None, :], (P, 1))).astype(np.float32)
    bor = np.ascontiguousarray(np.tile(bo[None, :], (P, 1))).astype(np.float32)

    # pad bias [P, 24]: col j<8 -> slot0 loop pos j; col 8+j -> slot1 loop pos j
    padb = np.zeros((P, 24), np.float32)
    valid = padding_mask[b]  # [S] bool
    for slot in range(2):
        qhi = qlo[slot] + 511
        for j, m in enumerate(SLOT_KTILES[slot]):
            g = perm[m * P:(m + 1) * P]
            ok = valid[g] & (g <= qhi)
            padb[:, (0 if slot == 0 else 8) + j] = np.where(ok, 0.0, NEG)

    # tri [P, 4, 512]: 1 if (t*128 + p) <= q else 0 (constant across cores)
    kk = np.arange(P)[:, None]
    qq = np.arange(512)[None, :]
    trib = np.stack([(t * P + kk) <= qq for t in range(4)], axis=1)
    trib = trib.astype(NPBF16)

    return {"xt": xt, "wq": wl(Wq), "wk": wl(Wk), "wv": wl(Wv), "wo": wl(Wo),
            "bqp": bqp, "bkp": bkp, "bvr": bvr, "bor": bor,
            "pad": padb, "tri": np.ascontiguousarray(trib)}, own


_NC_CACHE = {}


def kernel(x, padding_mask, Wq, bq, Wk, bk, Wv, bv, Wo, bo):
    x = np.asarray(x, np.float32)
    padding_mask = np.asarray(padding_mask, bool)
    args = [np.asarray(a, np.float32) for a in (Wq, bq, Wk, bk, Wv, bv, Wo, bo)]

    if "nc" not in _NC_CACHE:
        _NC_CACHE["nc"] = _build()
    nc = _NC_CACHE["nc"]

    in_maps, owns = [], []
    for c in range(8):
        m, own = _core_inputs(c, x, padding_mask, *args)
        in_maps.append(m)
        owns.append(own)

    trace = bool(int(os.environ.get("KERNEL_TRACE", "0")))
    res = run_bass_kernel_spmd(nc, in_maps, core_ids=list(range(8)), trace=trace)
    if trace:
        print(f"HW exec time: {res.exec_time_ns} ns")
        if res.instructions_and_trace is not None:
            print("trace:", res.instructions_and_trace[1])
        _NC_CACHE["exec_time_ns"] = res.exec_time_ns

    full = np.empty((B, S, D), np.float32)
    for c in range(8):
        full[c // 2, owns[c]] = res.results[c]["out"]
    return full


if __name__ == "__main__":
    rng = np.random.default_rng(0)
    x = rng.standard_normal((B, S, D), dtype=np.float32)
    lengths = rng.integers(S // 2, S + 1, size=(B,))
    pm = np.arange(S)[None, :] < lengths[:, None]
    std = 0.02
    ws = {n: (rng.standard_normal((D, D), dtype=np.float32) * std)
          for n in ("Wq", "Wk", "Wv", "Wo")}
    z = np.zeros((D,), np.float32)
    out = kernel(x, pm, ws["Wq"], z, ws["Wk"], z, ws["Wv"], z, ws["Wo"], z)
    print(out.shape, out.dtype, np.abs(out).mean())


# revision 30
# speedup vs baseline: 145.3342x; 145.3342x over previous
"""Causal multi-head attention (B=4, S=2048, D=1024, H=16) on 8 TRN2 NeuronCores.

Sharding: zero-collective. Core c handles batch b=c//2 and a causally-balanced
half of the queries (zigzag 256-token stripes: half 0 = stripes {0,3,4,7},
half 1 = {1,2,5,6} -- equal causal work). Each core computes Q for its 1024
tokens, K/V for the full sequence of its batch (duplicated across the pair of
cores -- cheaper than any collective), all 16 heads of causal attention, and
the output projection for its tokens. Host reassembles.

All 8 cores run one SPMD graph; per-core differences live only in DMA'd data.
The per-core key axis is permuted to [own tokens | other tokens] so the causal
diagonal lands at identical graph positions on every core; padding and
block-level causality enter via a per-core additive bias (per-partition,
folded into the ScalarE exp bias), and the within-diagonal triangle via a
constant 0/1 multiplicative mask.

Attention layout: transposed scores [k, q]. Per head pair and k-tile, two
N=512 score matmuls fill adjacent full PSUM banks, one fused ScalarE exp
(scale=1/8, bias=mask) reads across both banks and writes bf16 E; ctx
accumulates per pair (two heads col-packed in one bank); the softmax
denominator accumulates on DVE in bf16 with a final ones-matmul (M=64) that
both sums across partitions and broadcasts, feeding the reciprocal normalize.
Emission order pipelines attention segments against later QKV stages so the
ScalarE exp stream overlaps projection matmuls.
"""

import os
import sys

sys.path.insert(0, "/opt/trn_rl_repo")

import numpy as np
import ml_dtypes

import concourse.bass as bass
import concourse.bacc as bacc
import concourse.tile as tile
from concourse import mybir
from concourse.bass_utils import run_bass_kernel_spmd

B, S, D, H = 4, 2048, 1024, 16
HD = D // H  # 64
P = 128
NQ = S // 2  # queries per core (1024)
KC = D // P  # 8 contraction chunks
QW = 512     # query stripe width
NEG = -1e30
BF16 = mybir.dt.bfloat16
F32 = mybir.dt.float32
NPBF16 = ml_dtypes.bfloat16

# Query stripes (of width 512) owned by each half, in global order.
OWN_STRIPES = ([0, 3], [1, 2])

# k-tile positions (permuted token space) each slot's k-loop visits: the union
# over both cores of the not-entirely-masked positions. Own stripe i sits at
# ktiles {2i, 2i+1}; slot s's own tokens are ktiles {2s, 2s+1} (the diagonal).
SLOT_KTILES = [
    [0, 1, 2, 3, 8, 9, 10, 11],
    list(range(16)),
]
N_SLOTS = 2
PAD_COLS = sum(len(k) for k in SLOT_KTILES)  # 24


def _build():
    nc = bacc.Bacc()

    xt = nc.declare_dram_parameter("xt", [P, KC, S], BF16, isOutput=False)
    wq = nc.declare_dram_parameter("wq", [P, KC, D], BF16, isOutput=False)
    wk = nc.declare_dram_parameter("wk", [P, KC, D], BF16, isOutput=False)
    wv = nc.declare_dram_parameter("wv", [P, KC, D], BF16, isOutput=False)
    wo = nc.declare_dram_parameter("wo", [P, KC, D], BF16, isOutput=False)
    bqp = nc.declare_dram_parameter("bqp", [P, KC], F32, isOutput=False)
    bkp = nc.declare_dram_parameter("bkp", [P, KC], F32, isOutput=False)
    bvr = nc.declare_dram_parameter("bvr", [P, D], BF16, isOutput=False)
    bor = nc.declare_dram_parameter("bor", [P, D], BF16, isOutput=False)
    pad = nc.declare_dram_parameter("pad", [P, PAD_COLS], F32, isOutput=False)
    tri = nc.declare_dram_parameter("tri", [P, 4, QW], BF16, isOutput=False)
    out = nc.declare_dram_parameter("out", [NQ, D], F32, isOutput=True)

    from contextlib import ExitStack

    with tile.TileContext(nc) as tc, ExitStack() as ctx:
        wpool = ctx.enter_context(tc.tile_pool(name="wpool", bufs=1))
        xpool = ctx.enter_context(tc.tile_pool(name="xpool", bufs=2))
        bigpool = ctx.enter_context(tc.tile_pool(name="bigpool", bufs=1))
        epool = ctx.enter_context(tc.tile_pool(name="epool", bufs=4))
        dpool = ctx.enter_context(tc.tile_pool(name="dpool", bufs=2))
        spool = ctx.enter_context(tc.tile_pool(name="spool", bufs=2))
        pp_acc = ctx.enter_context(tc.tile_pool(name="pp_acc", bufs=2, space="PSUM"))
        pp_sc = ctx.enter_context(tc.tile_pool(name="pp_sc", bufs=2, space="PSUM"))
        pp_ctx = ctx.enter_context(tc.tile_pool(name="pp_ctx", bufs=2, space="PSUM"))

        # ---- constants into SBUF ----
        wq_s = wpool.tile([P, KC, D], BF16, tag="wq")
        wk_s = wpool.tile([P, KC, D], BF16, tag="wk")
        wv_s = wpool.tile([P, KC, D], BF16, tag="wv")
        wo_s = wpool.tile([P, KC, D], BF16, tag="wo")
        bq_s = wpool.tile([P, KC], F32, tag="bq")
        bk_s = wpool.tile([P, KC], F32, tag="bk")
        bv_s = wpool.tile([P, D], BF16, tag="bv")
        bo_s = wpool.tile([P, D], BF16, tag="bo")
        pad_s = wpool.tile([P, PAD_COLS], F32, tag="pad")
        tri_s = wpool.tile([P, 4, QW], BF16, tag="tri")
        ones_s = wpool.tile([P, HD], BF16, tag="ones")
        nc.vector.memset(ones_s[:], 1.0)

        # ---- big persistent activations ----
        qT_s = bigpool.tile([P, KC, NQ], BF16, tag="qT")     # [pairdims, pair, q]
        kT_s = bigpool.tile([P, KC, S], BF16, tag="kT")      # [pairdims, pair, k]
        v_s = bigpool.tile([P, S // P, D], BF16, tag="v")    # [k in tile, ktile, do]
        cT_s = bigpool.tile([P, KC, NQ], BF16, tag="cT")     # [pairdims, pair, q]

        def load_xt(st):
            ssl = slice(st * 512, (st + 1) * 512)
            xt_t = xpool.tile([P, KC, 512], BF16, tag="xt")
            nc.sync.dma_start(xt_t[:], xt[:, :, ssl])
            return xt_t

        def qkv_stage(st, xt_t):
            """Project permuted tokens [st*512, (st+1)*512)."""
            ssl = slice(st * 512, (st + 1) * 512)
            plan = []
            if st < 2:  # own tokens -> Q first (unblocks attention slots)
                plan.append((wq_s, bq_s, qT_s))
            plan.append((wk_s, bk_s, kT_s))
            for w_s, b_s, dst in plan:
                for m in range(KC):
                    ps = pp_acc.tile([P, 512], F32, tag="acc")
                    for kc in range(KC):
                        nc.tensor.matmul(
                            ps[:], lhsT=w_s[:, kc, m * P:(m + 1) * P],
                            rhs=xt_t[:, kc, :],
                            start=(kc == 0), stop=(kc == KC - 1))
                    nc.vector.tensor_scalar_add(dst[:, m, ssl], ps[:], b_s[:, m:m + 1])
            for sub in range(4):
                for dt in range(2):
                    dsl = slice(dt * 512, (dt + 1) * 512)
                    ps = pp_acc.tile([P, 512], F32, tag="acc")
                    for kc in range(KC):
                        nc.tensor.matmul(
                            ps[:], lhsT=xt_t[:, kc, sub * P:(sub + 1) * P],
                            rhs=wv_s[:, kc, dsl],
                            start=(kc == 0), stop=(kc == KC - 1))
                    nc.vector.tensor_tensor(
                        v_s[:, st * 4 + sub, dsl], ps[:], bv_s[:, dsl],
                        mybir.AluOpType.add)

        pad_base = [0, 8]  # running offset of SLOT_KTILES lengths

        attn_state = {}

        def attn_pairs(slot, pairs, jlo, jhi):
            """Emit k-loop segment [jlo, jhi) of the given head pairs; the
            (ctx, dacc) accumulators live in attn_state across segments."""
            qsl = slice(slot * QW, (slot + 1) * QW)
            ktiles = SLOT_KTILES[slot]
            nkt = len(ktiles)
            for pr in pairs:
                hA, hB = 2 * pr, 2 * pr + 1
                if jlo == 0:
                    ctx_new = pp_ctx.tile([P, QW], F32, tag="ctx", name="ctx_ps")
                    dacc_new = dpool.tile([P, 2 * QW], BF16, tag="dacc", name="dacc")
                    attn_state[(slot, pr)] = (ctx_new, dacc_new)
                ctx_ps, dacc = attn_state[(slot, pr)]
                for j in range(jlo, jhi):
                    m = ktiles[j]
                    ksl = slice(m * P, (m + 1) * P)
                    # scores for both heads in adjacent full banks
                    sc = pp_sc.tile([P, 2 * QW], F32, tag="sc")
                    for q_i in range(2):
                        lo = q_i * HD
                        nc.tensor.matmul(
                            sc[:, q_i * QW:(q_i + 1) * QW],
                            lhsT=kT_s[lo:lo + HD, pr, ksl],
                            rhs=qT_s[lo:lo + HD, pr, qsl],
                            start=True, stop=True, tile_position=(lo, 0))
                    pcol = pad_base[slot] + j
                    e = epool.tile([P, 2 * QW], BF16, tag="e")
                    nc.scalar.activation(e[:], sc[:],
                                         mybir.ActivationFunctionType.Exp,
                                         bias=pad_s[:, pcol:pcol + 1], scale=0.125)
                    if 4 * slot <= m < 4 * slot + 4:  # diagonal band
                        for q_i in range(2):
                            esl = slice(q_i * QW, (q_i + 1) * QW)
                            nc.vector.tensor_tensor(e[:, esl], e[:, esl],
                                                    tri_s[:, m - 4 * slot, :],
                                                    mybir.AluOpType.mult)
                    if j == 0:
                        nc.vector.tensor_scalar_add(dacc[:], e[:], 0.0)
                    else:
                        nc.vector.tensor_tensor(dacc[:], dacc[:], e[:],
                                                mybir.AluOpType.add)
                    st_, sp_ = (j == 0), (j == nkt - 1)
                    for q_i, h in enumerate((hA, hB)):
                        lo = q_i * HD
                        nc.tensor.matmul(
                            ctx_ps[lo:lo + HD, :],
                            lhsT=v_s[:, m, h * HD:(h + 1) * HD],
                            rhs=e[:, q_i * QW:(q_i + 1) * QW],
                            start=st_, stop=sp_, tile_position=(0, lo),
                            skip_group_check=True)
                if jhi == nkt:
                    # denominator: broadcast column sums across partitions
                    den_ps = pp_sc.tile([P, 2 * QW], F32, tag="sc")
                    for q_i in range(2):
                        lo = q_i * HD
                        nc.tensor.matmul(
                            den_ps[lo:lo + HD, 0:QW],
                            lhsT=ones_s[:],
                            rhs=dacc[:, q_i * QW:(q_i + 1) * QW],
                            start=True, stop=True, tile_position=(0, lo),
                            skip_group_check=True)
                    rden = spool.tile([P, QW], F32, tag="rden")
                    nc.vector.reciprocal(rden[:], den_ps[:, 0:QW])
                    nc.vector.tensor_tensor(cT_s[:, pr, qsl], ctx_ps[:], rden[:],
                                            mybir.AluOpType.mult)
                    del attn_state[(slot, pr)]

        def oproj(st8):
            osl = slice(st8 * P, (st8 + 1) * P)
            for dt in range(2):
                dsl = slice(dt * 512, (dt + 1) * 512)
                ps = pp_acc.tile([P, 512], F32, tag="acc")
                for kc in range(KC):
                    nc.tensor.matmul(ps[:], lhsT=cT_s[:, kc, osl],
                                     rhs=wo_s[:, kc, dsl],
                                     start=(kc == 0), stop=(kc == KC - 1))
                ob = spool.tile([P, 512], F32, tag="outsb")
                nc.vector.tensor_tensor(ob[:], ps[:], bo_s[:, dsl],
                                        mybir.AluOpType.add)
                nc.sync.dma_start(out[osl, dsl], ob[:])

        nc.sync.dma_start(wq_s[:], wq[:])
        nc.sync.dma_start(wk_s[:], wk[:])
        xt0 = load_xt(0)
        nc.sync.dma_start(wv_s[:], wv[:])
        nc.sync.dma_start(bq_s[:], bqp[:])
        nc.sync.dma_start(bk_s[:], bkp[:])
        nc.sync.dma_start(bv_s[:], bvr[:])
        nc.sync.dma_start(pad_s[:], pad[:])
        nc.sync.dma_start(tri_s[:], tri[:])
        qkv_stage(0, xt0)
        xt2 = load_xt(2)
        nc.sync.dma_start(wo_s[:], wo[:])
        nc.sync.dma_start(bo_s[:], bor[:])
        qkv_stage(2, xt2)
        attn_pairs(0, list(range(8)), 0, 8)
        xt1 = load_xt(1)
        qkv_stage(1, xt1)
        attn_pairs(1, [0, 1], 0, 12)     # all but stage-3 k-tiles
        xt3 = load_xt(3)
        qkv_stage(3, xt3)
        attn_pairs(1, [0, 1], 12, 16)
        oproj(0)
        oproj(1)
        attn_pairs(1, [2, 3], 0, 16)
        oproj(2)
        oproj(3)
        attn_pairs(1, [4, 5], 0, 16)
        attn_pairs(1, [6, 7], 0, 16)
        for st8 in range(4, 8):
            oproj(st8)

    nc.compile()
    return nc


def _stripe_tokens(stripes):
    return np.concatenate([np.arange(s * QW, (s + 1) * QW) for s in stripes])


def _core_inputs(c, x, padding_mask, Wq, bq, Wk, bk, Wv, bv, Wo, bo):
    b, h = c // 2, c % 2
    own_stripes = OWN_STRIPES[h]
    rest_stripes = [s for s in range(S // QW) if s not in own_stripes]
    own = _stripe_tokens(own_stripes)
    rest = _stripe_tokens(rest_stripes)
    perm = np.concatenate([own, rest])
    qlo = [s * QW for s in own_stripes]  # global start of slot s's queries

    xt = np.ascontiguousarray(
        x[b][perm].T.reshape(KC, P, S).transpose(1, 0, 2)).astype(NPBF16)

    def wl(W):
        return np.ascontiguousarray(
            W.T.reshape(KC, P, D).transpose(1, 0, 2)).astype(NPBF16)

    bqp = np.ascontiguousarray(bq.reshape(KC, P).T).astype(np.float32)
    bkp = np.ascontiguousarray(bk.reshape(KC, P).T).astype(np.float32)
    bvr = np.ascontiguousarray(np.tile(bv[None, :], (P, 1))).astype(NPBF16)
    bor = np.ascontiguousarray(np.tile(bo[None, :], (P, 1))).astype(NPBF16)

    # pad bias [P, 40]: per (slot, loop position): 0 where key is valid
    # (unpadded and key-stripe not entirely after the queries), else -1e30.
    # The within-diagonal triangle is handled by `tri`, so diagonal tiles get
    # padding-only here.
    padb = np.zeros((P, PAD_COLS), np.float32)
    valid = padding_mask[b]  # [S] bool
    col = 0
    for slot in range(N_SLOTS):
        qhi = qlo[slot] + QW - 1
        for m in SLOT_KTILES[slot]:
            g = perm[m * P:(m + 1) * P]
            ok = valid[g] & (g <= qhi)
            padb[:, col] = np.where(ok, 0.0, NEG)
            col += 1

    # tri [P, 4, QW]: shift t in {0..3}: 1 if (t*128 + p) <= q_rel else 0
    # (applied per fused half; same for all cores).
    kk = np.arange(P)[:, None]
    qq = np.arange(QW)[None, :]
    trib = np.stack([(t * P + kk) <= qq for t in range(4)], axis=1).astype(NPBF16)

    return {"xt": xt, "wq": wl(Wq), "wk": wl(Wk), "wv": wl(Wv), "wo": wl(Wo),
            "bqp": bqp, "bkp": bkp, "bvr": bvr, "bor": bor,
            "pad": padb, "tri": np.ascontiguousarray(trib)}, own


_NC_CACHE = {}


def kernel(x, padding_mask, Wq, bq, Wk, bk, Wv, bv, Wo, bo):
    x = np.asarray(x, np.float32)
    padding_mask = np.asarray(padding_mask, bool)
    args = [np.asarray(a, np.float32) for a in (Wq, bq, Wk, bk, Wv, bv, Wo, bo)]

    if "nc" not in _NC_CACHE:
        _NC_CACHE["nc"] = _build()
    nc = _NC_CACHE["nc"]

    in_maps, owns = [], []
    for c in range(8):
        m, own = _core_inputs(c, x, padding_mask, *args)
        in_maps.append(m)
        owns.append(own)

    trace = bool(int(os.environ.get("KERNEL_TRACE", "0")))
    try:
        res = run_bass_kernel_spmd(nc, in_maps, core_ids=list(range(8)), trace=trace)
    except ModuleNotFoundError:
        # NTFF profiling hook unavailable in this environment
        res = run_bass_kernel_spmd(nc, in_maps, core_ids=list(range(8)))
    if trace and res.exec_time_ns is not None:
        print(f"HW exec time: {res.exec_time_ns} ns")
        _NC_CACHE["exec_time_ns"] = res.exec_time_ns

    full = np.empty((B, S, D), np.float32)
    for c in range(8):
        full[c // 2, owns[c]] = res.results[c]["out"]
    return full


if __name__ == "__main__":
    rng = np.random.default_rng(0)
    x = rng.standard_normal((B, S, D), dtype=np.float32)
    lengths = rng.integers(S // 2, S + 1, size=(B,))
    pm = np.arange(S)[None, :] < lengths[:, None]
    std = 0.02
    ws = {n: (rng.standard_normal((D, D), dtype=np.float32) * std)
          for n in ("Wq", "Wk", "Wv", "Wo")}
    z = np.zeros((D,), np.float32)
    out = kernel(x, pm, ws["Wq"], z, ws["Wk"], z, ws["Wv"], z, ws["Wo"], z)
    print(out.shape, out.dtype, np.abs(out).mean())
